# revision 25
# baseline (speedup 1.0000x reference)
"""Trainium2 Bass kernel for nn_DrugResponsePrior (embedding_lookup).

Spec guarantees: cell_map < 100, is_missing in {0,1}, drug_map < 256.  So each
row's result depends only on cs = cell_map[idx]+100*is_missing[idx] (200
states) and dm = drug_map[tidx] (256 drugs).

Fully data-parallel strategy (8 cores, 8192 samples each, no collectives):
  1. Build CS = cell_map + 100*is_missing as a uint8 table (and drug_map as
     uint8) in DRAM; reload them in a 16-slab SBUF layout (partition p holds
     entries [(p%16)*16384, ...)).
  2. Look up cs/dm per sample with GPSIMD indirect_copy (per-16-partition
     group index lists, offsets = idx & 16383); resolve the 16-way slab
     ambiguity with a one-hot mask (built from idx>>14 via a small
     group-broadcast matmul + iota compare) and a group-reduce matmul.
  3. Precompute A = l2n(cell-embedding table) @ Wf1c + bf1 ([200,200]) and
     Bd = l2n(drug_emb) @ Wf1d ([256,200]) once.
  4. Per sample, build one-hot matrices over cs (200) / dm (256) and run the
     MLP with matmuls: h1 = relu(A^T Sc + Bd^T Sd), h2 = relu(Wf2^T h1 + bf2),
     fwd = Wf3^T h2 (bias via constant-ones row), mu = cumsum-of-softplus via
     a 9x9 matmul.

All matmul operands are bf16 (PE runs 1 cycle/row vs 4 for fp32); PSUM
accumulation stays fp32, norms and softplus stay fp32-ish.  Measured numpy
emulation of this quantization: rel_fro ~ 6e-4 (gate is 2e-2).

All reference math runs on device; the host only reshapes/transposes/casts
parameters and slices idx/tidx (pure index arithmetic: & 16383, >> 14).
"""
import sys

if "/opt/trn_rl_repo" not in sys.path:
    sys.path.insert(0, "/opt/trn_rl_repo")

import numpy as np
import ml_dtypes

import concourse.bass as bass
import concourse.mybir as mybir
import concourse.tile as tile
from concourse.bass_utils import run_bass_kernel_spmd

f32 = mybir.dt.float32
bf16 = mybir.dt.bfloat16
i32 = mybir.dt.int32
u16 = mybir.dt.uint16
u8 = mybir.dt.uint8
nbf = ml_dtypes.bfloat16

B = 65536
R = 262144
NDRUG = 256
NFEAT = 1024
CEMB = 1024
DEMB = 128
HID = 200
NDOSES = 9
NCORES = 8

BS = B // NCORES            # 8192 samples per core
P = 128
NG = 8                      # gpsimd groups (16 partitions each)
GS = BS // NG               # 1024 samples per group
SLAB = R // 16              # 16384 entries per slab partition
NCHUNK = BS // 512          # 16 chunks of 512 samples
EPS = 1e-12

_NC_CACHE = {}


def _split_sync_waits(nc, limit=1):
    """This walrus accepts at most one sync-wait per instruction; hoist excess
    waits onto same-engine NoOps inserted just before."""
    ctr = 0
    for bb in nc.main_func.blocks:
        new_list = []
        for inst in bb.instructions:
            si = inst.sync_info
            if si is not None and si.on_wait and len(si.on_wait) > limit:
                waits = list(si.on_wait)
                head, tail = waits[:-limit], waits[-limit:]
                for j in range(0, len(head), limit):
                    nop = mybir.InstNoOp(name=f"waitnop-{ctr}", engine=inst.engine)
                    ctr += 1
                    nop.sync_info = mybir.SyncInfo(
                        on_wait=list(head[j : j + limit]), on_update=[]
                    )
                    new_list.append(nop)
                inst.sync_info = mybir.SyncInfo(
                    on_wait=list(tail),
                    on_update=list(si.on_update) if si.on_update else [],
                )
            new_list.append(inst)
        bb.instructions[:] = new_list
    return nc


def build_nc(split_waits=True):
    nc = bass.Bass(num_devices=NCORES)
    AF = mybir.ActivationFunctionType
    ALU = mybir.AluOpType

    # ---------------- kernel I/O ----------------
    u_idx = nc.dram_tensor("u_idx", [P, GS // 16], u16, kind="ExternalInput")
    u_tidx = nc.dram_tensor("u_tidx", [P, GS // 16], u16, kind="ExternalInput")
    q_idx = nc.dram_tensor("q_idx", [NG, GS], bf16, kind="ExternalInput")
    q_tidx = nc.dram_tensor("q_tidx", [NG, GS], bf16, kind="ExternalInput")
    cell_map = nc.dram_tensor("cell_map", [R], u8, kind="ExternalInput")
    is_missing = nc.dram_tensor("is_missing", [R], u8, kind="ExternalInput")
    drug_map = nc.dram_tensor("drug_map", [R], u8, kind="ExternalInput")
    cfTp = nc.dram_tensor("cfTp", [P, 8 * 100], bf16, kind="ExternalInput")
    me_in = nc.dram_tensor("me_in", [100, CEMB], f32, kind="ExternalInput")
    drug_emb = nc.dram_tensor("drug_emb", [NDRUG, DEMB], f32, kind="ExternalInput")
    drug_embT = nc.dram_tensor("drug_embT", [DEMB, NDRUG], bf16, kind="ExternalInput")
    W1p = nc.dram_tensor("W1p", [P, 8 * CEMB], bf16, kind="ExternalInput")
    b1 = nc.dram_tensor("b1", [CEMB], bf16, kind="ExternalInput")
    Wf1cp = nc.dram_tensor("Wf1cp", [P, 8 * HID], bf16, kind="ExternalInput")
    Wf1d = nc.dram_tensor("Wf1d", [DEMB, HID], bf16, kind="ExternalInput")
    bf1 = nc.dram_tensor("bf1", [HID], bf16, kind="ExternalInput")
    Wf2 = nc.dram_tensor("Wf2", [HID, HID], bf16, kind="ExternalInput")
    bf2 = nc.dram_tensor("bf2", [HID], f32, kind="ExternalInput")
    Wf3k0 = nc.dram_tensor("Wf3k0", [P, NDOSES], bf16, kind="ExternalInput")
    # rows 0..71 = Wf3[128:200]; 72..95 zero; row 96 = bf3 (bias folded
    # via a constant ones row at partition 96 — engine writes must start at
    # partition 0/32/64/96)
    Wf3k1 = nc.dram_tensor("Wf3k1", [97, NDOSES], bf16, kind="ExternalInput")
    mu_sT = nc.dram_tensor("mu_sT", [NDOSES, BS], f32, kind="ExternalOutput")

    # inline constants (input-value independent)
    grp_bc = nc.inline_tensor(  # [8, 128] group-broadcast lhsT
        np.array([[1.0 if (m // 16) == g else 0.0 for m in range(P)]
                  for g in range(NG)], nbf), name="grp_bc")
    grp_rd = nc.inline_tensor(  # [128, 8] group-reduce lhsT
        np.array([[1.0 if (k // 16) == g else 0.0 for g in range(NG)]
                  for k in range(P)], nbf), name="grp_rd")
    qi_const = nc.inline_tensor(
        (np.arange(P, dtype=np.float32).reshape(P, 1) % 16), name="qi_const")
    iota_colc = nc.inline_tensor(
        np.arange(P, dtype=np.float32).reshape(P, 1), name="iota_colc")
    # cumsum matrix rows 0..7 = softplus doses (L[k,o]=1 iff k+1<=o), row 8 = base
    L9np = np.concatenate(
        [np.triu(np.ones((NDOSES - 1, NDOSES), np.float32), 1),
         np.ones((1, NDOSES), np.float32)], axis=0).astype(nbf)
    L9 = nc.inline_tensor(L9np, name="L9")

    # internal DRAM for the combined-state uint8 table
    CS8 = nc.dram_tensor("CS8", [R], u8)

    with tile.TileContext(nc) as tc, \
            tc.tile_pool(name="sb", bufs=1) as sb, \
            tc.tile_pool(name="sbw", bufs=1) as sbw:

        # ======== index lists first (tiny, gate piece0) ========
        u_idx_sb = sb.tile([P, GS // 16], u16)
        nc.sync.dma_start(out=u_idx_sb[:], in_=u_idx[:])
        u_tidx_sb = sb.tile([P, GS // 16], u16)
        nc.sync.dma_start(out=u_tidx_sb[:], in_=u_tidx[:])
        q_idx_sb = sb.tile([NG, GS], bf16)
        nc.sync.dma_start(out=q_idx_sb[:], in_=q_idx[:])
        q_tidx_sb = sb.tile([NG, GS], bf16)
        nc.sync.dma_start(out=q_tidx_sb[:], in_=q_tidx[:])

        # ======== slab path: it gates the lookup pipeline ========
        # slab halves ride the gpsimd queue (SWDGE) + one half each on
        # sync/scalar; the gpsimd queue then runs the indirect gathers, so
        # ordering is natural
        slab_cm = tc.tile_pool(name="sb_slab", bufs=1)
        slab_pool = slab_cm.__enter__()
        cs_slab = slab_pool.tile([P, SLAB], u8)
        dm_slab = slab_pool.tile([P, SLAB], u8)
        half_ap = lambda t: bass.AP(tensor=t, offset=0,
                                    ap=[[0, NG // 2], [SLAB, 16], [1, SLAB]])
        nc.sync.dma_start(out=dm_slab[0:64, :], in_=half_ap(drug_map.ap().tensor))
        with tc.tile_pool(name="sbx", bufs=1) as sbx:
            cmv = sbx.tile([P, R // P], u8)
            miv = sbx.tile([P, R // P], u8)
            nc.sync.dma_start(out=cmv[:], in_=cell_map[:].rearrange("(p a) -> p a", p=P))
            nc.sync.dma_start(out=miv[:], in_=is_missing[:].rearrange("(p a) -> p a", p=P))
            nc.vector.tensor_scalar_mul(out=miv[:], in0=miv[:], scalar1=100)
            nc.vector.tensor_tensor(out=cmv[:], in0=cmv[:], in1=miv[:], op=ALU.add)
            nc.sync.dma_start(out=CS8[:].rearrange("(p a) -> p a", p=P), in_=cmv[:])
        nc.sync.dma_start(out=cs_slab[0:64, :], in_=half_ap(CS8.ap().tensor))

        # ======== constants / small params ========
        grp_bc_sb = sbw.tile([NG, P], bf16)
        nc.sync.dma_start(out=grp_bc_sb[:], in_=grp_bc[:])
        grp_rd_sb = sbw.tile([P, NG], bf16)
        nc.sync.dma_start(out=grp_rd_sb[:], in_=grp_rd[:])
        qi_sb = sbw.tile([P, 1], f32)
        nc.sync.dma_start(out=qi_sb[:], in_=qi_const[:])
        qi32_sb = sbw.tile([P, 1], f32)
        nc.sync.dma_start(out=qi32_sb[:], in_=iota_colc[:])
        L9_sb = sbw.tile([NDOSES, NDOSES], bf16)
        nc.sync.dma_start(out=L9_sb[:], in_=L9[:])
        ones_c100 = sbw.tile([1, 100], bf16)
        nc.vector.memset(ones_c100[:], 1.0)
        ones_c128 = sbw.tile([1, P], bf16)
        nc.vector.memset(ones_c128[:], 1.0)

        # ======== params (host-packed k-tile layouts; split across queues) ====
        cft_all = sbw.tile([P, 8 * 100], bf16)
        nc.scalar.dma_start(out=cft_all[:], in_=cfTp[:])
        w1_all = sbw.tile([P, 8 * CEMB], bf16)   # [p, (k n)]
        nc.scalar.dma_start(out=w1_all[:, 0:4 * CEMB], in_=W1p[:, 0:4 * CEMB])
        # slab second halves mid-scalar: they gate piece0
        nc.scalar.dma_start(out=cs_slab[64:128, :], in_=half_ap(CS8.ap().tensor))
        nc.scalar.dma_start(out=dm_slab[64:128, :], in_=half_ap(drug_map.ap().tensor))
        nc.scalar.dma_start(out=w1_all[:, 4 * CEMB:], in_=W1p[:, 4 * CEMB:])
        wf1c_all = sbw.tile([P, 8 * HID], bf16)
        nc.sync.dma_start(out=wf1c_all[:], in_=Wf1cp[:])
        me_sb = sb.tile([100, CEMB], f32)
        nc.sync.dma_start(out=me_sb[:], in_=me_in[:])
        b1_row = sbw.tile([1, CEMB], bf16)
        nc.sync.dma_start(out=b1_row[:], in_=b1[:].rearrange("(one n) -> one n", one=1))
        bf1_row = sbw.tile([1, HID], bf16)
        nc.sync.dma_start(out=bf1_row[:], in_=bf1[:].rearrange("(one n) -> one n", one=1))
        wf1d_sb = sbw.tile([DEMB, HID], bf16)
        nc.sync.dma_start(out=wf1d_sb[:], in_=Wf1d[:])
        wf2_k0 = sbw.tile([P, HID], bf16)
        wf2_k1 = sbw.tile([HID - P, HID], bf16)
        nc.sync.dma_start(out=wf2_k0[:], in_=Wf2[0:P, :])
        nc.sync.dma_start(out=wf2_k1[:], in_=Wf2[P:HID, :])
        wf3_k0 = sbw.tile([P, NDOSES], bf16)
        wf3_k1 = sbw.tile([97, NDOSES], bf16)
        nc.sync.dma_start(out=wf3_k0[:], in_=Wf3k0[:])
        nc.sync.dma_start(out=wf3_k1[:], in_=Wf3k1[:])
        bf2_c0 = sbw.tile([P, 1], f32)
        bf2_c1 = sbw.tile([HID - P, 1], f32)
        nc.sync.dma_start(out=bf2_c0[:], in_=bf2[0:P].rearrange("(p one) -> p one", one=1))
        nc.sync.dma_start(out=bf2_c1[:], in_=bf2[P:HID].rearrange("(p one) -> p one", one=1))
        de_p = []
        for mt in range(2):
            t = sb.tile([P, DEMB], f32, tag=f"de_{mt}")
            nc.sync.dma_start(out=t[:], in_=drug_emb[mt * P:(mt + 1) * P, :])
            de_p.append(t)
        deT_sb = sb.tile([DEMB, NDRUG], bf16)
        nc.sync.dma_start(out=deT_sb[:], in_=drug_embT[:])

        # ======== table construction: A [200,200], Bd [256,200] ========
        with (
            tc.tile_pool(name="ps_tr", bufs=3, space="PSUM") as ps_tr,
            tc.tile_pool(name="ps_ps", bufs=2, space="PSUM") as ps_ps,
            tc.tile_pool(name="ps_aa", bufs=1, space="PSUM") as ps_aa,
        ):
            from concourse.masks import make_identity
            ident = sbw.tile([P, P], f32)
            make_identity(nc, ident[:])

            # P100 = relu(cf @ W1 + b1)  [100, 1024]
            p_sb = sb.tile([100, CEMB], f32)
            for nh in range(2):
                pps = ps_ps.tile([100, 512], f32, tag="pshard")
                for kt in range(8):
                    nc.tensor.matmul(
                        out=pps[:], lhsT=cft_all[:, kt * 100:(kt + 1) * 100],
                        rhs=w1_all[:, kt * CEMB + nh * 512:kt * CEMB + (nh + 1) * 512],
                        start=(kt == 0), stop=False)
                nc.tensor.matmul(
                    out=pps[:], lhsT=ones_c100[:],
                    rhs=b1_row[:, nh * 512:(nh + 1) * 512], start=False, stop=True)
                nc.scalar.activation(
                    out=p_sb[:, nh * 512:(nh + 1) * 512], in_=pps[:], func=AF.Relu)

            # norm scales, scaled rows
            sq = sb.tile([100, CEMB], f32)
            ssp = sb.tile([100, 1], f32)
            ssm = sb.tile([100, 1], f32)
            nc.scalar.activation(out=sq[:], in_=p_sb[:], func=AF.Square)
            nc.vector.reduce_sum(out=ssp[:], in_=sq[:], axis=mybir.AxisListType.X)
            nc.scalar.activation(out=sq[:], in_=me_sb[:], func=AF.Square)
            nc.vector.reduce_sum(out=ssm[:], in_=sq[:], axis=mybir.AxisListType.X)
            for ss in (ssp, ssm):
                nc.scalar.activation(out=ss[:], in_=ss[:], func=AF.Sqrt)
                nc.vector.tensor_scalar_max(out=ss[:], in0=ss[:], scalar1=EPS)
                nc.vector.reciprocal(out=ss[:], in_=ss[:])
            nc.vector.tensor_scalar_mul(out=p_sb[:], in0=p_sb[:], scalar1=ssp[:])
            nc.vector.tensor_scalar_mul(out=me_sb[:], in0=me_sb[:], scalar1=ssm[:])
            cp_sb, cm_sb = p_sb, me_sb

            # CnT k-tiles [128, 200] (cols: 100 present + 100 missing), bf16
            cnt_kt = []
            for kt in range(8):
                t = sb.tile([P, 2 * 100], bf16, tag=f"cnt_{kt}")
                for (src, co) in ((cp_sb, 0), (cm_sb, 100)):
                    tp = ps_tr.tile([P, 100], f32, tag="tr")
                    nc.tensor.transpose(
                        out=tp[:], in_=src[:, kt * P:(kt + 1) * P],
                        identity=ident[:100, :100])
                    nc.vector.tensor_copy(out=t[:, co:co + 100], in_=tp[:])
                cnt_kt.append(t)

            # A k-tiles (states on partitions): A0 [128, 200], A1 [72, 200]
            a_k = []
            for (mt, msl) in ((0, slice(0, P)), (1, slice(P, HID))):
                mm = msl.stop - msl.start
                aps = ps_aa.tile([P, HID], f32, tag="a")
                for kt in range(8):
                    nc.tensor.matmul(
                        out=aps[:mm, :], lhsT=cnt_kt[kt][:, msl],
                        rhs=wf1c_all[:, kt * HID:(kt + 1) * HID],
                        start=(kt == 0), stop=False)
                nc.tensor.matmul(
                    out=aps[:mm, :], lhsT=ones_c128[:, :mm], rhs=bf1_row[:],
                    start=False, stop=True)
                t = sb.tile([mm, HID], bf16, tag=f"a_{mt}")
                nc.vector.tensor_copy(out=t[:], in_=aps[:mm, :])
                a_k.append(t)

            # drug: rd scales + Bd k-tiles [128, 200] x2 (drugs on partitions)
            bd_k = []
            for mt in range(2):
                sqd = sb.tile([P, DEMB], f32, tag="sqd")
                rd = sb.tile([P, 1], f32, tag=f"rd_{mt}")
                nc.scalar.activation(out=sqd[:], in_=de_p[mt][:], func=AF.Square)
                nc.vector.reduce_sum(out=rd[:], in_=sqd[:], axis=mybir.AxisListType.X)
                nc.scalar.activation(out=rd[:], in_=rd[:], func=AF.Sqrt)
                nc.vector.tensor_scalar_max(out=rd[:], in0=rd[:], scalar1=EPS)
                nc.vector.reciprocal(out=rd[:], in_=rd[:])
                bps = ps_aa.tile([P, HID], f32, tag="bd")
                nc.tensor.matmul(out=bps[:], lhsT=deT_sb[:, mt * P:(mt + 1) * P],
                                 rhs=wf1d_sb[:], start=True, stop=True)
                t = sb.tile([P, HID], bf16, tag=f"bd_{mt}")
                nc.vector.tensor_scalar_mul(out=t[:], in0=bps[:], scalar1=rd[:])
                bd_k.append(t)

        # ======== phase 2+3: lookup pieces (emitted interleaved with chunks) ========
        g_cs = sb.tile([P, GS], u8)
        g_dm = sb.tile([P, GS], u8)
        sel_cm = tc.tile_pool(name="ps_sel", bufs=1, space="PSUM")
        ps_sel = sel_cm.__enter__()
        selsb_cm = tc.tile_pool(name="sb_sel", bufs=2)
        sb_sel = selsb_cm.__enter__()
        v8_cs = sb.tile([NG, GS], bf16, tag="v8_cs")
        v8_dm = sb.tile([NG, GS], bf16, tag="v8_dm")
        v8 = {"cs": v8_cs, "dm": v8_dm}

        def emit_piece(t):
            jsl = slice(t * 512, (t + 1) * 512)
            isl = slice(t * 32, (t + 1) * 32)
            for (name, gt, qt, ut, slab) in (
                    ("cs", g_cs, q_idx_sb, u_idx_sb, cs_slab),
                    ("dm", g_dm, q_tidx_sb, u_tidx_sb, dm_slab)):
                nc.gpsimd.indirect_copy(
                    out=gt[:, jsl].rearrange("p (n one) -> p n one", one=1),
                    data=slab[:], idxs=ut[:, isl],
                    i_know_ap_gather_is_preferred=True)
                qmask = sb_sel.tile([P, 512], bf16, tag="qmask")
                qb = ps_sel.tile([P, 512], f32, tag="selps")
                nc.tensor.matmul(out=qb[:], lhsT=grp_bc_sb[:], rhs=qt[:, jsl],
                                 start=True, stop=True)
                nc.vector.tensor_scalar(
                    out=qmask[:], in0=qb[:], scalar1=qi_sb[:], scalar2=None,
                    op0=ALU.is_equal)
                gf = sb_sel.tile([P, 512], bf16, tag="gf")
                nc.vector.tensor_copy(out=gf[:], in_=gt[:, jsl])
                nc.vector.tensor_tensor(out=gf[:], in0=gf[:], in1=qmask[:],
                                        op=ALU.mult)
                vp = ps_sel.tile([NG, 512], f32, tag="selps")
                nc.tensor.matmul(out=vp[:], lhsT=grp_rd_sb[:], rhs=gf[:],
                                 start=True, stop=True)
                nc.vector.tensor_copy(out=v8[name][:, jsl], in_=vp[:])

        # ======== per-sample chunk pipeline ========
        with (
            tc.tile_pool(name="ps_hA", bufs=1, space="PSUM") as ps_hA,
            tc.tile_pool(name="ps_hB", bufs=1, space="PSUM") as ps_hB,
            tc.tile_pool(name="ps_hC", bufs=1, space="PSUM") as ps_hC,
            tc.tile_pool(name="sbc", bufs=2) as sbc,
        ):
            gb_full = sb.tile([NDOSES, BS], bf16)    # rows 0..7 softplus, row 8 base
            mu_sbT = sb.tile([NDOSES, BS], f32)      # mu, dose-major
            # h2 m-tile 1 double buffers with a persistent ones row (72) for
            # the folded f9 bias
            h2s1_bufs = []
            for i in range(2):
                t = sbw.tile([97, 512], bf16, tag=f"h2s1_{i}")
                nc.vector.memset(t[64:97, :], 0.0)
                nc.vector.memset(t[96:97, :], 1.0)
                h2s1_bufs.append(t)

            masks = {}

            def emit_front_dma(ch):
                g = ch // 2
                j0 = (ch % 2) * 512
                # codes for chunk ch: 1-descriptor SBUF->SBUF DMA of the v8
                # row (engine ops can't read partition g directly)
                bcrow = sbc.tile([1, 512], bf16, tag="bcrow")
                nc.sync.dma_start(out=bcrow[:], in_=v8_cs[g:g + 1, j0:j0 + 512])
                bdrow = sbc.tile([1, 512], bf16, tag="bdrow")
                nc.sync.dma_start(out=bdrow[:], in_=v8_dm[g:g + 1, j0:j0 + 512])
                return bcrow, bdrow

            def emit_front_mm(ch, rows):
                # K=1 matmul broadcasts codes across partitions into PSUM;
                # masks are built two chunks ahead of their consumption so the
                # PE pipeline never waits on them
                bcrow, bdrow = rows
                bcp = ps_hA.tile([P, 512], f32, tag="bcast")
                nc.tensor.matmul(out=bcp[:], lhsT=ones_c128[:], rhs=bcrow[:],
                                 start=True, stop=True)
                sc0 = sbc.tile([P, 512], bf16, tag="sc0")
                sc1 = sbc.tile([HID - P, 512], bf16, tag="sc1")
                nc.vector.tensor_scalar(out=sc0[:], in0=bcp[:], scalar1=qi32_sb[:],
                                        scalar2=None, op0=ALU.is_equal)
                nc.vector.tensor_scalar(out=sc1[:], in0=bcp[:HID - P, :],
                                        scalar1=128.0, scalar2=qi32_sb[:HID - P, :],
                                        op0=ALU.subtract, op1=ALU.is_equal)
                bdp = ps_hA.tile([P, 512], f32, tag="bdcast")
                nc.tensor.matmul(out=bdp[:], lhsT=ones_c128[:], rhs=bdrow[:],
                                 start=True, stop=True)
                sd0 = sbc.tile([P, 512], bf16, tag="sd0")
                sd1 = sbc.tile([P, 512], bf16, tag="sd1")
                nc.vector.tensor_scalar(out=sd0[:], in0=bdp[:], scalar1=qi32_sb[:],
                                        scalar2=None, op0=ALU.is_equal)
                nc.vector.tensor_scalar(out=sd1[:], in0=bdp[:], scalar1=128.0,
                                        scalar2=qi32_sb[:], op0=ALU.subtract,
                                        op1=ALU.is_equal)
                masks[ch] = (sc0, sc1, sd0, sd1)

            def emit_chunk(ch, post_h1=None, post_h2=None):
                n0 = ch * 512
                sl = slice(n0, n0 + 512)
                sc0, sc1, sd0, sd1 = masks.pop(ch)
                # h1 = relu(A^T Sc + Bd^T Sd)   [200, 512]
                h1s = []
                for (mt, msl) in ((0, slice(0, P)), (1, slice(P, HID))):
                    mm = msl.stop - msl.start
                    hp = ps_hA.tile([P, 512], f32, tag=f"h1_{mt}")
                    nc.tensor.matmul(out=hp[:mm, :], lhsT=a_k[0][:, msl], rhs=sc0[:],
                                     start=True, stop=False)
                    nc.tensor.matmul(out=hp[:mm, :], lhsT=a_k[1][:, msl], rhs=sc1[:],
                                     start=False, stop=False)
                    nc.tensor.matmul(out=hp[:mm, :], lhsT=bd_k[0][:, msl], rhs=sd0[:],
                                     start=False, stop=False)
                    nc.tensor.matmul(out=hp[:mm, :], lhsT=bd_k[1][:, msl], rhs=sd1[:],
                                     start=False, stop=True)
                    hs = sbc.tile([mm, 512], bf16, tag=f"h1s_{mt}")
                    nc.scalar.activation(out=hs[:], in_=hp[:mm, :], func=AF.Relu)
                    h1s.append(hs)
                if post_h1 is not None:
                    post_h1()
                # h2 = relu(Wf2^T h1 + bf2)   [200, 512]; m-tile 1 lands in
                # h2s1_bufs rows 0..71 (row 72 is the constant ones row)
                h2s1 = h2s1_bufs[ch % 2]
                hp0 = ps_hB.tile([P, 512], f32, tag="h2_0")
                nc.tensor.matmul(out=hp0[:], lhsT=wf2_k0[:, 0:P], rhs=h1s[0][:],
                                 start=True, stop=False)
                nc.tensor.matmul(out=hp0[:], lhsT=wf2_k1[:, 0:P], rhs=h1s[1][:],
                                 start=False, stop=True)
                h2s0 = sbc.tile([P, 512], bf16, tag="h2s_0")
                nc.scalar.activation(out=h2s0[:], in_=hp0[:], func=AF.Relu,
                                     bias=bf2_c0[:], scale=1.0)
                hp1 = ps_hB.tile([P, 512], f32, tag="h2_1")
                nc.tensor.matmul(out=hp1[:HID - P, :], lhsT=wf2_k0[:, P:HID],
                                 rhs=h1s[0][:], start=True, stop=False)
                nc.tensor.matmul(out=hp1[:HID - P, :], lhsT=wf2_k1[:, P:HID],
                                 rhs=h1s[1][:], start=False, stop=True)
                nc.scalar.activation(out=h2s1[0:HID - P, :], in_=hp1[:HID - P, :],
                                     func=AF.Relu, bias=bf2_c1[:], scale=1.0)
                if post_h2 is not None:
                    post_h2()
                # all 9 fwd doses in one psum; host permuted Wf3 columns to
                # [dose1..dose8, dose0] so softplus rows start at partition 0;
                # bf3 rides on the constant ones row of h2s1
                f9 = ps_hC.tile([NDOSES, 512], f32, tag="fbmu")
                nc.tensor.matmul(out=f9[:], lhsT=wf3_k0[:], rhs=h2s0[:],
                                 start=True, stop=False)
                nc.tensor.matmul(out=f9[:], lhsT=wf3_k1[:], rhs=h2s1[:],
                                 start=False, stop=True)
                # softplus rows 0..7 (one fused act), base row 8 via DVE copy
                # softplus = ln(exp(x)+1) (Softplus isn't in the compiler's
                # act tables; Exp+Ln share one table set with Relu)
                nc.scalar.activation(out=gb_full[0:NDOSES - 1, sl],
                                     in_=f9[:NDOSES - 1, :], func=AF.Exp)
                nc.scalar.activation(out=gb_full[0:NDOSES - 1, sl],
                                     in_=gb_full[0:NDOSES - 1, sl],
                                     func=AF.Ln, bias=1.0, scale=1.0)
                s9 = sbc.tile([NDOSES, 512], bf16, tag="s9")
                nc.vector.tensor_copy(out=s9[:], in_=f9[:])
                nc.sync.dma_start(out=gb_full[NDOSES - 1:NDOSES, sl],
                                  in_=s9[NDOSES - 1:NDOSES, :])
            def emit_mu(ch):
                # mu for chunk ch, dose-major: muT[o, s] = sum_k L9[k, o]
                # gb[k, s] — ONE N=512 matmul instead of four 9-column ones.
                # Deferred two chunks behind the main pipeline so the PE never
                # waits on the gb row-8 DMA round trip.
                n0 = ch * 512
                sl = slice(n0, n0 + 512)
                mups = ps_hB.tile([NDOSES, 512], f32, tag="h2_0")
                nc.tensor.matmul(out=mups[:], lhsT=L9_sb[:],
                                 rhs=gb_full[:, sl], start=True, stop=True)
                nc.vector.tensor_copy(out=mu_sbT[:, sl], in_=mups[:])
                nc.sync.dma_start(out=mu_sT[:, sl], in_=mu_sbT[:, sl])

            from collections import deque
            pending = deque()

            emit_piece(0)
            emit_piece(1)
            # chunks grouped by the piece that feeds them (evens use piece0,
            # odds piece1) so no chunk ever waits on a later lookup piece
            order = list(range(0, NCHUNK, 2)) + list(range(1, NCHUNK, 2))
            rows = {}
            rows[order[0]] = emit_front_dma(order[0])
            rows[order[1]] = emit_front_dma(order[1])
            emit_front_mm(order[0], rows.pop(order[0]))
            emit_front_mm(order[1], rows.pop(order[1]))
            for i, ch in enumerate(order):
                if i + 2 < NCHUNK:
                    rows[order[i + 2]] = emit_front_dma(order[i + 2])
                post_h1 = (
                    (lambda c=order[i + 2]: emit_front_mm(c, rows.pop(c)))
                    if i + 2 < NCHUNK else None)
                post_h2 = (
                    (lambda: emit_mu(pending.popleft()))
                    if len(pending) >= 2 else None)
                emit_chunk(ch, post_h1=post_h1, post_h2=post_h2)
                pending.append(ch)
            while pending:
                emit_mu(pending.popleft())
        selsb_cm.__exit__(None, None, None)
        sel_cm.__exit__(None, None, None)
        slab_cm.__exit__(None, None, None)

    return _split_sync_waits(nc) if split_waits else nc


def _get_nc():
    if "nc" not in _NC_CACHE:
        _NC_CACHE["nc"] = build_nc()
    return _NC_CACHE["nc"]


def make_in_maps(inputs):
    idx = np.asarray(inputs["idx"], np.int64)
    tidx = np.asarray(inputs["tidx"], np.int64)
    cf = np.asarray(inputs["cell_features"], np.float32)
    me = np.asarray(inputs["missing_emb"], np.float32)
    de = np.asarray(inputs["drug_emb"], np.float32)
    Wf1 = np.asarray(inputs["Wf1"], np.float32)
    Wf3 = np.asarray(inputs["Wf3"], np.float32)[:, [1, 2, 3, 4, 5, 6, 7, 8, 0]]
    bf3 = np.asarray(inputs["bf3"], np.float32)[[1, 2, 3, 4, 5, 6, 7, 8, 0]]

    cbf = lambda a: np.ascontiguousarray(a.astype(nbf))
    def kpack(a):
        # [8*128, n] -> [128, 8*n] with tile kt at columns [kt*n, (kt+1)*n)
        n = a.shape[1]
        return np.ascontiguousarray(
            a.reshape(8, P, n).transpose(1, 0, 2).reshape(P, 8 * n))
    shared = dict(
        cell_map=np.ascontiguousarray(np.asarray(inputs["cell_map"]).astype(np.uint8)),
        is_missing=np.ascontiguousarray(np.asarray(inputs["is_missing"]).astype(np.uint8)),
        drug_map=np.ascontiguousarray(np.asarray(inputs["drug_map"]).astype(np.uint8)),
        cfTp=cbf(kpack(cf[:100, :].T.astype(np.float32))),
        me_in=np.ascontiguousarray(me),
        drug_emb=np.ascontiguousarray(de),
        drug_embT=cbf(de.T),
        W1p=cbf(kpack(np.asarray(inputs["W1"], np.float32))),
        b1=cbf(np.asarray(inputs["b1"], np.float32)),
        Wf1cp=cbf(kpack(Wf1[:CEMB, :])),
        Wf1d=cbf(Wf1[CEMB:, :]),
        bf1=cbf(np.asarray(inputs["bf1"], np.float32)),
        Wf2=cbf(np.asarray(inputs["Wf2"], np.float32)),
        bf2=np.ascontiguousarray(np.asarray(inputs["bf2"], np.float32)),
        Wf3k0=cbf(Wf3[0:P, :]),
        Wf3k1=cbf(np.concatenate(
            [Wf3[P:HID, :], np.zeros((24, NDOSES), np.float32), bf3[None, :]],
            axis=0)),
    )

    def wrap16(vals):
        # vals [8192] in sample order k (g = k>>10, j = k&1023)
        # -> [128, 64] at [16g + (j & 15), j >> 4]
        v = vals.reshape(NG, GS // 16, 16)        # [g, j_hi, j_lo]
        v = np.transpose(v, (0, 2, 1))            # [g, j_lo, j_hi]
        return np.ascontiguousarray(v.reshape(P, GS // 16))

    in_maps = []
    for c in range(NCORES):
        ic = idx[c * BS:(c + 1) * BS]
        tc_ = tidx[c * BS:(c + 1) * BS]
        m = dict(shared)
        m["u_idx"] = wrap16((ic & (SLAB - 1)).astype(np.uint16))
        m["u_tidx"] = wrap16((tc_ & (SLAB - 1)).astype(np.uint16))
        m["q_idx"] = np.ascontiguousarray(
            (ic >> 14).astype(nbf).reshape(NG, GS))
        m["q_tidx"] = np.ascontiguousarray(
            (tc_ >> 14).astype(nbf).reshape(NG, GS))
        in_maps.append(m)
    return in_maps


def kernel(**inputs):
    nc = _get_nc()
    in_maps = make_in_maps(inputs)
    last_err = None
    for _attempt in range(3):
        try:
            res = run_bass_kernel_spmd(nc, in_maps, core_ids=list(range(NCORES)))
            return np.concatenate(
                [np.ascontiguousarray(res.results[c]["mu_sT"].T)
                 for c in range(NCORES)], axis=0)
        except Exception as e:  # wedged device sometimes recovers on retry
            last_err = e
    raise last_err
